# revision 28
# baseline (speedup 1.0000x reference)
"""Trainium2 Bass kernel for nn_BDH_69638599737422 (dense_transformer).

Sharding (8 NeuronCores): core c = 2*h + j owns head h (of 4) and N-half j
(4096 of 8192 latent dims). encoder/encoder_v column-parallel, decoder
row-parallel. Per layer: one 2-rank AllReduce (partial yKV within a head
pair, since scores contract over the full head N) and one 8-rank AllReduce
(y = xy @ decoder partial sums into D).

All on-device tensors are fp16 (PE matmuls run fp16 at full rate with fp32
PSUM accumulation; verified ~1.3e-3 rel err vs the fp32 reference).

The RoPE frequency table repeats in pairs (quantize(t,2)), so a host-side
even/odd de-interleave permutation of each core's N slice (baked into
encoder/encoder_v columns and decoder rows) turns rotate_half into two
contiguous halves: qe = xe*c - xo*s, qo = xo*c + xe*s.

The causal mask (strict lower) is applied on the transposed score matrix
P[s,t] = scores[t,s]: Q@Q^T is symmetric, so P comes out of the same
matmuls and the mask becomes strict-upper, which lets the per-s-chunk
matmuls skip the all-zero left region entirely (triangle skip).
"""

import numpy as np

import concourse.bass as bass
import concourse.tile as tile
from concourse import bacc, mybir
from concourse.bass_utils import run_bass_kernel_spmd
from concourse.masks import make_identity

F16 = mybir.dt.float16
BF16 = mybir.dt.bfloat16
F32 = mybir.dt.float32
AF = mybir.ActivationFunctionType
ALU = mybir.AluOpType

B, T, D, NH, VOCAB = 1, 512, 256, 4, 256
N = 8192        # latent dim per head
NL = 4096       # per-core N slice
NPAIR = 2048    # rope pairs per core
NT = NL // 128  # 32 n-tiles per core
N_LAYER = 6
EPS = 1e-5
THETA = 2.0 ** 16
TWO_PI = 2.0 * np.pi
CORES = list(range(8))
PAIR_GROUPS = [[0, 1], [2, 3], [4, 5], [6, 7]]

_STATE = {}


# ---------------------------------------------------------------- host prep

def _ln_np(x):
    m = x.mean(-1, keepdims=True)
    v = ((x - m) ** 2).mean(-1, keepdims=True)
    return (x - m) / np.sqrt(v + EPS)


def _rope_pair_tables():
    """cos/sin at even lanes only (freqs repeat in pairs): [T, N//2] f32."""
    t = np.arange(N, dtype=np.float32)
    q = (np.floor(t / 2.0) * 2.0).astype(np.float32)
    freqs = (1.0 / (THETA ** (q / np.float32(N))) / np.float32(TWO_PI)).astype(
        np.float32
    )
    pos = np.arange(T, dtype=np.float32)
    ang = ((pos[:, None] * freqs[None, :]) % 1.0) * np.float32(TWO_PI)
    cos = np.cos(ang).astype(np.float32)
    sin = np.sin(ang).astype(np.float32)
    return cos[:, ::2], sin[:, ::2]


def _tileize_rows(a, rows_per_tile=128):
    """[n_tiles*128, w] -> [128, n_tiles*w] with free dim = (tile, w)."""
    r, w = a.shape
    nt = r // rows_per_tile
    return np.ascontiguousarray(
        a.reshape(nt, rows_per_tile, w).transpose(1, 0, 2).reshape(rows_per_tile, nt * w)
    )


def _build_in_maps(idx, embed, encoder, encoder_v, decoder, lm_head):
    idx = np.asarray(idx)
    embed = np.asarray(embed, dtype=np.float32)
    encoder = np.asarray(encoder, dtype=np.float32)
    encoder_v = np.asarray(encoder_v, dtype=np.float32)
    decoder = np.asarray(decoder, dtype=np.float32)
    lm_head = np.asarray(lm_head, dtype=np.float32)

    x0 = _ln_np(embed[idx[0]]).astype(np.float16)          # [T, D]
    x_td0 = _tileize_rows(x0)                               # [128, 4*256]
    x_dt0 = _tileize_rows(np.ascontiguousarray(x0.T))       # [128, 2*512]

    cos_p, sin_p = _rope_pair_tables()                      # [T, 4096] f32
    # even lanes first, then odd lanes
    perm = np.concatenate([np.arange(0, NL, 2), np.arange(1, NL, 2)])

    maskd = np.triu(np.ones((128, 128), np.float16), k=1)   # keep s < t
    lmh = _tileize_rows(lm_head.astype(np.float16))         # [128, 2*256]

    in_maps = []
    for c in CORES:
        h, j = c // 2, c % 2
        nsl = slice(j * NL, (j + 1) * NL)
        enc_s = encoder[h][:, nsl][:, perm].astype(np.float16)      # [256, 4096]
        ev_s = encoder_v[h][:, nsl][:, perm].astype(np.float16)
        dec_s = decoder[h * N + j * NL : h * N + (j + 1) * NL][perm].astype(
            np.float16
        )                                                            # [4096, 256]
        kp = slice(j * NPAIR, (j + 1) * NPAIR)
        cos_s = np.ascontiguousarray(cos_p[:, kp].T).astype(np.float16)  # [2048, 512]
        sin_s = np.ascontiguousarray(sin_p[:, kp].T).astype(np.float16)
        in_maps.append(
            {
                "enc0": np.ascontiguousarray(enc_s[:128]),
                "enc1": np.ascontiguousarray(enc_s[128:]),
                "ev0": np.ascontiguousarray(ev_s[:128]),
                "ev1": np.ascontiguousarray(ev_s[128:]),
                "decb": _tileize_rows(dec_s),               # [128, 32*256]
                "cosb": _tileize_rows(cos_s),               # [128, 16*512]
                "sinb": _tileize_rows(sin_s),
                "maskd": maskd,
                "x_td0": x_td0,
                "x_dt0": x_dt0,
                "lmh": lmh,
            }
        )
    return in_maps


# ---------------------------------------------------------------- device code

def _ln_chunk(nc, st, out_f16, in_ap, tc, chunk, epst):
    """LN over one free-dim chunk: out = (in - mu) * rstd."""
    sl = slice(tc * chunk, (tc + 1) * chunk)
    stats = st.tile([128, 6], F32, tag="st6", name="stats")
    mv = st.tile([128, 2], F32, tag="st2", name="mv")
    nc.vector.bn_stats(out=stats, in_=in_ap[:, sl])
    nc.vector.bn_aggr(out=mv, in_=stats)
    nc.scalar.activation(
        out=mv[:, 1:2], in_=mv[:, 1:2], func=AF.Sqrt, bias=epst, scale=1.0
    )
    nc.vector.reciprocal(out=mv[:, 1:2], in_=mv[:, 1:2])
    nc.vector.tensor_scalar(
        out=out_f16[:, sl],
        in0=in_ap[:, sl],
        scalar1=mv[:, 0:1],
        scalar2=mv[:, 1:2],
        op0=ALU.subtract,
        op1=ALU.mult,
    )


def _transpose_blocks(nc, ps, dst, src, n_tc, n_dc, ident):
    """dst[(dc,t-block)] = src[(tc,d-block)]^T for [128,128] blocks.

    src free = (tc, n_dc*128), dst free = (dc, n_tc*128)."""
    for tc in range(n_tc):
        for dc in range(n_dc):
            tr = ps.tile([128, 128], F16, tag="ps")
            nc.tensor.transpose(
                tr, src[:, tc * (n_dc * 128) + dc * 128 :][:, :128], ident
            )
            nc.scalar.copy(
                out=dst[:, dc * (n_tc * 128) + tc * 128 :][:, :128], in_=tr
            )


def _build_bass():
    nc = bacc.Bacc(None, target_bir_lowering=False, num_devices=len(CORES))

    dp = nc.declare_dram_parameter
    enc0_e = dp("enc0", [128, NL], F16, isOutput=False)
    enc1_e = dp("enc1", [128, NL], F16, isOutput=False)
    ev0_e = dp("ev0", [128, NL], F16, isOutput=False)
    ev1_e = dp("ev1", [128, NL], F16, isOutput=False)
    dec_e = dp("decb", [128, NT * D], F16, isOutput=False)
    cos_e = dp("cosb", [128, 16 * T], F16, isOutput=False)
    sin_e = dp("sinb", [128, 16 * T], F16, isOutput=False)
    mask_e = dp("maskd", [128, 128], F16, isOutput=False)
    xtd_e = dp("x_td0", [128, 4 * D], F16, isOutput=False)
    xdt_e = dp("x_dt0", [128, 2 * T], F16, isOutput=False)
    lmh_e = dp("lmh", [128, 2 * VOCAB], F16, isOutput=False)
    out_e = dp("logits", [T, VOCAB], F32, isOutput=True)

    with tile.TileContext(nc) as tc_:
        pools = [
            tc_.tile_pool(name="wt", bufs=1),
            tc_.tile_pool(name="big", bufs=1),
            tc_.tile_pool(name="xp", bufs=2),
            tc_.tile_pool(name="tmp", bufs=1),
            tc_.tile_pool(name="ys", bufs=3),
            tc_.tile_pool(name="st", bufs=8),
            tc_.tile_pool(name="stg", bufs=1),
            tc_.tile_pool(name="ps", bufs=8, space="PSUM"),
            tc_.tile_pool(name="dram", bufs=2, space="DRAM"),
        ]
        wt, big, xp, tmp, ysp, st, stg, ps, dram = [p.__enter__() for p in pools]
        try:
            _emit(nc, wt, big, xp, tmp, ysp, st, stg, ps, dram,
                  enc0_e, enc1_e, ev0_e, ev1_e, dec_e, cos_e, sin_e, mask_e,
                  xtd_e, xdt_e, lmh_e, out_e)
        finally:
            for p in reversed(pools):
                p.__exit__(None, None, None)
    nc.compile()
    return nc


def _emit(nc, wt, big, xp, tmp, ysp, st, stg, ps, dram,
          enc0_e, enc1_e, ev0_e, ev1_e, dec_e, cos_e, sin_e, mask_e,
          xtd_e, xdt_e, lmh_e, out_e):
    dma = nc.sync.dma_start

    # persistent weights / tables
    enc0 = wt.tile([128, NL], F16, tag="enc0")
    enc1 = wt.tile([128, NL], F16, tag="enc1")
    ev0 = wt.tile([128, NL], F16, tag="ev0")
    ev1 = wt.tile([128, NL], F16, tag="ev1")
    dect = wt.tile([128, NT * D], F16, tag="dect")
    cost = wt.tile([128, 16 * T], F16, tag="cost")
    sint = wt.tile([128, 16 * T], F16, tag="sint")
    maskt = wt.tile([128, 128], F16, tag="maskt")
    lmht = wt.tile([128, 2 * VOCAB], F16, tag="lmht")
    ident = wt.tile([128, 128], F16, tag="ident")
    epst = wt.tile([128, 1], F32, tag="epst")

    xsb = big.tile([128, NT * T], F16, tag="xsb")    # xs then xy, (i, t)
    qrb = big.tile([128, NT * T], F16, tag="qrb")    # roped qs, (i, t)
    Pb = big.tile([128, 4 * T], F16, tag="Pb")       # masked scores^T, (sc, t)

    x_first = xp.tile([128, 4 * D], F16, tag="x_td")
    xd_first = xp.tile([128, 2 * T], F16, tag="x_dt")
    dma(out=x_first, in_=xtd_e[:])
    dma(out=xd_first, in_=xdt_e[:])
    dma(out=enc0, in_=enc0_e[:])
    dma(out=enc1, in_=enc1_e[:])
    dma(out=cost, in_=cos_e[:])
    dma(out=sint, in_=sin_e[:])
    dma(out=maskt, in_=mask_e[:])
    dma(out=ev0, in_=ev0_e[:])
    dma(out=ev1, in_=ev1_e[:])
    dma(out=dect, in_=dec_e[:])
    dma(out=lmht, in_=lmh_e[:])
    nc.vector.memset(epst, EPS)
    make_identity(nc, ident[:])

    # zero regions of P (left of the diagonal block) are written once;
    # every layer only rewrites the diagonal + upper blocks
    for m in range(1, 4):
        nc.vector.memset(Pb[:, m * T : m * T + m * 128], 0.0)

    # warm up the collective path (first-call setup costs ~30us) while the
    # weight DMAs stream in; outputs are never consumed.
    wup = stg.tile([128, 2 * D], F16, tag="wup")
    nc.vector.memset(wup, 0.0)
    wag_i = dram.tile([128, D], F32, tag="wag_i")
    wag_o = dram.tile([2, 128, D], F32, tag="wag_o")
    war_i = dram.tile([128, 2 * D], F16, tag="war_i")
    war_o = dram.tile([128, 2 * D], F16, tag="war_o")
    dma(out=wag_i[:].bitcast(F16), in_=wup)
    dma(out=war_i, in_=wup)
    nc.gpsimd.collective_compute(
        "AllGather", ALU.bypass, replica_groups=PAIR_GROUPS,
        ins=[wag_i.opt()], outs=[wag_o.opt()],
    )
    nc.gpsimd.collective_compute(
        "AllReduce", ALU.add, replica_groups=[CORES],
        ins=[war_i.opt()], outs=[war_o.opt()],
    )

    # phase 1: xs = relu(x @ enc), out [nl, t] tiles. t-halved (so it can
    # slide into the previous layer's AR2 windows) and (even, odd) lane
    # tiles produced pairwise so rope chunks unblock early. Relu copies
    # alternate ACT/DVE to keep either from becoming the feeder bottleneck.
    order = [k for p in zip(range(16), range(16, NT)) for k in p]

    def phase1_half(h, x_dt_ap):
        hsl = slice(h * 256, h * 256 + 256)
        for n_i, i in enumerate(order):
            mm = ps.tile([128, 256], F32, tag="ps", name="mm1")
            nc.tensor.matmul(
                out=mm, lhsT=enc0[:, i * 128 : (i + 1) * 128],
                rhs=x_dt_ap[:, 0 * T : 1 * T][:, hsl], start=True, stop=False,
            )
            nc.tensor.matmul(
                out=mm, lhsT=enc1[:, i * 128 : (i + 1) * 128],
                rhs=x_dt_ap[:, 1 * T : 2 * T][:, hsl], start=False, stop=True,
            )
            dst = xsb[:, i * T : (i + 1) * T][:, hsl]
            if n_i < 8:
                nc.vector.tensor_relu(out=dst, in_=mm)
            else:
                nc.scalar.activation(out=dst, in_=mm, func=AF.Relu)

    x_td, x_dt = x_first, xd_first
    phase1_half(0, x_dt)
    phase1_half(1, x_dt)
    for _layer in range(N_LAYER):
        # -- phase 2: rope, 4 chunks of 4 tile-pairs, all on DVE (GpSimd
        # sharing the reads poisons DVE SBUF ports — measured 4.4x slowdown).
        # qe = xe*c - xo*s, qo = xo*c + xe*s; TT multiply runs 2x, the
        # combine is a fused scalar_tensor_tensor ((v * -+1) + u) at 1x.
        CH = 4 * T  # 2048 columns per chunk
        for c in range(4):
            e_sl = slice(c * CH, (c + 1) * CH)
            o_sl = slice(16 * T + c * CH, 16 * T + (c + 1) * CH)
            tme = tmp.tile([128, CH], F16, tag="tmpe")
            nc.vector.tensor_mul(tme, xsb[:, o_sl], sint[:, e_sl])
            nc.vector.tensor_mul(qrb[:, e_sl], xsb[:, e_sl], cost[:, e_sl])
            nc.vector.scalar_tensor_tensor(
                out=qrb[:, e_sl], in0=tme, scalar=-1.0, in1=qrb[:, e_sl],
                op0=ALU.mult, op1=ALU.add,
            )
            tmo = tmp.tile([128, CH], F16, tag="tmpo")
            nc.vector.tensor_mul(tmo, xsb[:, e_sl], sint[:, e_sl])
            nc.vector.tensor_mul(qrb[:, o_sl], xsb[:, o_sl], cost[:, e_sl])
            nc.vector.scalar_tensor_tensor(
                out=qrb[:, o_sl], in0=tmo, scalar=1.0, in1=qrb[:, o_sl],
                op0=ALU.mult, op1=ALU.add,
            )

        # -- phase 3: P[s,t] = (qr^T qr) masked to s < t (triangle skip).
        # jt streamed in rope-chunk completion order so PE consumption
        # tracks DVE production.
        P_ps = [ps.tile([128, T], F32, tag="ps", name=f"P_ps{m}") for m in range(4)]
        jt_stream = []
        for c in range(4):
            jt_stream += [4 * c + k for k in range(4)]
            jt_stream += [16 + 4 * c + k for k in range(4)]
        for idx_jt, jt in enumerate(jt_stream):
            base = jt * T
            for m in range(4):
                t0 = m * 128
                nc.tensor.matmul(
                    out=P_ps[m][:, t0:T],
                    lhsT=qrb[:, base + t0 : base + t0 + 128],
                    rhs=qrb[:, base + t0 : base + T],
                    start=(idx_jt == 0), stop=(idx_jt == NT - 1),
                    skip_group_check=True,
                )
        for m in range(4):
            t0 = m * 128
            nc.vector.tensor_mul(
                Pb[:, m * T + t0 : m * T + t0 + 128],
                P_ps[m][:, t0 : t0 + 128],
                maskt,
            )
            if m < 3:
                nc.scalar.copy(
                    out=Pb[:, m * T + t0 + 128 : (m + 1) * T],
                    in_=P_ps[m][:, t0 + 128 : T],
                )

        # -- phase 4: yKV partial = P^T-contraction with x (V); the pair
        # reduce is a bypass AllGather (fp16-safe, low floor) + local add,
        # t-halved so the h1 exchange overlaps the h0 tail + phase 5 h0.
        ykv_ps = [ps.tile([128, D], F32, tag="ps", name=f"ykv_ps{m}") for m in range(4)]
        stage1 = stg.tile([128, 4 * D], F16, tag="stg1")
        b1o = []
        for h in range(2):
            for k in range(2):
                tcn = 2 * h + k
                for sc in range(4):
                    nc.tensor.matmul(
                        out=ykv_ps[tcn],
                        lhsT=Pb[:, sc * T + tcn * 128 : sc * T + (tcn + 1) * 128],
                        rhs=x_td[:, sc * D : (sc + 1) * D],
                        start=(sc == 0), stop=(sc == 3),
                        skip_group_check=True,
                    )
                # 1/64 pre-scale keeps the pair-sum inside fp16 range; the
                # LN that follows is scale-invariant so this is exact.
                nc.vector.tensor_scalar_mul(
                    out=stage1[:, tcn * D : (tcn + 1) * D],
                    in0=ykv_ps[tcn],
                    scalar1=0.015625,
                )
            b1i_h = dram.tile([128, D], F32, tag=f"b1i{h}", name=f"b1i{h}")
            b1o_h = dram.tile(
                [2, 128, D], F32, tag=f"b1o{h}", name=f"b1o{h}"
            )
            dma(
                out=b1i_h[:].bitcast(F16),
                in_=stage1[:, h * 2 * D : (h + 1) * 2 * D],
            )
            nc.gpsimd.collective_compute(
                "AllGather", ALU.bypass, replica_groups=PAIR_GROUPS,
                ins=[b1i_h.opt()], outs=[b1o_h.opt()],
            )
            b1o.append(b1o_h)

        ykvsum = stg.tile([128, 4 * D], F16, tag="ykvsum")
        agt = stg.tile([128, 2 * D], F16, tag="agt")
        ykv_td = xp.tile([128, 4 * D], F16, tag="ykv_td")
        ykv_dt = xp.tile([128, 2 * T], F16, tag="ykv_dt")
        y_ps = [ps.tile([128, D], F32, tag="ps", name=f"y_ps{m}") for m in range(4)]
        stage2 = stg.tile([128, 4 * D], F16, tag="stg2")
        b2o = []
        for h in range(2):
            hsl = slice(h * 2 * D, (h + 1) * 2 * D)
            dma(out=ykvsum[:, hsl], in_=b1o[h][0].bitcast(F16))
            dma(out=agt, in_=b1o[h][1].bitcast(F16))
            nc.vector.tensor_add(ykvsum[:, hsl], ykvsum[:, hsl], agt)
            for k in range(2):
                tcn = 2 * h + k
                _ln_chunk(nc, st, ykv_td, ykvsum, tcn, D, epst)
                for dc in range(2):
                    tr = ps.tile([128, 128], F16, tag="ps", name="tr")
                    nc.tensor.transpose(
                        tr, ykv_td[:, tcn * D + dc * 128 :][:, :128], ident
                    )
                    nc.scalar.copy(
                        out=ykv_dt[:, dc * T + tcn * 128 :][:, :128], in_=tr
                    )

            # phase 5 half: ys = relu(yKV @ encv); xy = xs*ys; y += xy @ dec.
            # xy runs once per 4-tile group through a strided 3D AP to
            # amortize DVE op overhead.
            for g in range(NT // 4):
                ys_grp = ysp.tile([128, 4 * 256], F16, tag="ys", name="ys_grp")
                for k4 in range(4):
                    i = 4 * g + k4
                    mm = ps.tile([128, 256], F32, tag="ps", name="mm5")
                    nc.tensor.matmul(
                        out=mm, lhsT=ev0[:, i * 128 : (i + 1) * 128],
                        rhs=ykv_dt[:, 0 * T + h * 256 :][:, :256],
                        start=True, stop=False,
                    )
                    nc.tensor.matmul(
                        out=mm, lhsT=ev1[:, i * 128 : (i + 1) * 128],
                        rhs=ykv_dt[:, 1 * T + h * 256 :][:, :256],
                        start=False, stop=True,
                    )
                    nc.scalar.activation(
                        out=ys_grp[:, k4 * 256 : (k4 + 1) * 256], in_=mm,
                        func=AF.Relu,
                    )
                xs_grp = (
                    xsb[:, 4 * g * T : 4 * (g + 1) * T]
                    .rearrange("p (i t) -> p i t", t=T)[:, :, h * 256 : (h + 1) * 256]
                )
                nc.vector.tensor_mul(
                    xs_grp, xs_grp,
                    ys_grp.rearrange("p (i t) -> p i t", t=256),
                )
                for k4 in range(4):
                    i = 4 * g + k4
                    for k in range(2):
                        tcn = 2 * h + k
                        nc.tensor.matmul(
                            out=y_ps[tcn],
                            lhsT=xsb[:, i * T + tcn * 128 : i * T + (tcn + 1) * 128],
                            rhs=dect[:, i * D : (i + 1) * D],
                            start=(i == 0), stop=(i == NT - 1),
                            skip_group_check=True,
                        )

            # 8-core AllReduce of this half's y partials
            for k in range(2):
                tcn = 2 * h + k
                nc.vector.tensor_copy(
                    out=stage2[:, tcn * D : (tcn + 1) * D], in_=y_ps[tcn]
                )
            b2i_h = dram.tile([128, 2 * D], F16, tag=f"b2i{h}", name=f"b2i{h}")
            b2o_h = dram.tile([128, 2 * D], F16, tag=f"b2o{h}", name=f"b2o{h}")
            dma(out=b2i_h, in_=stage2[:, hsl])
            nc.gpsimd.collective_compute(
                "AllReduce", ALU.add, replica_groups=[CORES],
                ins=[b2i_h.opt()], outs=[b2o_h.opt()],
            )
            b2o.append(b2o_h)

        # -- phase 7: y = LN(ysum); x = LN(x + y); refresh x_dt (per half)
        ysum = stg.tile([128, 4 * D], F16, tag="ysum")
        y_ln = xp.tile([128, 4 * D], F16, tag="y_ln")
        z = xp.tile([128, 4 * D], F16, tag="z")
        x_td_new = xp.tile([128, 4 * D], F16, tag="x_td")
        x_dt_new = xp.tile([128, 2 * T], F16, tag="x_dt")
        for h in range(2):
            hsl = slice(h * 2 * D, (h + 1) * 2 * D)
            dma(out=ysum[:, hsl], in_=b2o[h])
            for k in range(2):
                tcn = 2 * h + k
                _ln_chunk(nc, st, y_ln, ysum, tcn, D, epst)
            nc.vector.scalar_tensor_tensor(
                out=z[:, hsl], in0=y_ln[:, hsl], scalar=1.0, in1=x_td[:, hsl],
                op0=ALU.mult, op1=ALU.add,
            )
            for k in range(2):
                tcn = 2 * h + k
                _ln_chunk(nc, st, x_td_new, z, tcn, D, epst)
                for dc in range(2):
                    tr = ps.tile([128, 128], F16, tag="ps", name="trx")
                    nc.tensor.transpose(
                        tr, x_td_new[:, tcn * D + dc * 128 :][:, :128], ident
                    )
                    nc.scalar.copy(
                        out=x_dt_new[:, dc * T + tcn * 128 :][:, :128], in_=tr
                    )
            # next layer's phase-1 half rides in this AR2/tail window
            if _layer < N_LAYER - 1:
                phase1_half(h, x_dt_new)
            else:
                for k in range(2):
                    tcn = 2 * h + k
                    lg = ps.tile([128, VOCAB], F32, tag="ps", name="lg")
                    for dc in range(2):
                        nc.tensor.matmul(
                            out=lg,
                            lhsT=x_dt_new[:, dc * T + tcn * 128 : dc * T + (tcn + 1) * 128],
                            rhs=lmht[:, dc * VOCAB : (dc + 1) * VOCAB],
                            start=(dc == 0), stop=(dc == 1),
                        )
                    lg_sb = ysp.tile([128, VOCAB], F32, tag="lg", name="lg_sb")
                    nc.vector.tensor_copy(out=lg_sb, in_=lg)
                    dma(out=out_e[tcn * 128 : (tcn + 1) * 128, :], in_=lg_sb)
        x_td, x_dt = x_td_new, x_dt_new



# ---------------------------------------------------------------- entry point

def kernel(idx, embed, encoder, encoder_v, decoder, lm_head):
    if "nc" not in _STATE:
        _STATE["nc"] = _build_bass()
    nc = _STATE["nc"]
    in_maps = _build_in_maps(idx, embed, encoder, encoder_v, decoder, lm_head)
    import os

    trace = bool(int(os.environ.get("KERNEL_TRACE", "0")))
    res = run_bass_kernel_spmd(nc, in_maps, core_ids=CORES, trace=trace)
    _STATE["last_results"] = res
    return res.results[0]["logits"].reshape(B, T, VOCAB).astype(np.float32)
